# revision 3
# baseline (speedup 1.0000x reference)
"""Trainium2 Bass kernel for a 2-layer GCN + link predictor (PrimeKG drug
repurposing GNN).

Strategy (8 NeuronCores, SPMD single program):
  - Nodes are permuted into 128-node "buckets" balanced by in-degree; each
    core owns NBLK/8 consecutive buckets (rows of the aggregation).
  - Edges are grouped by destination bucket and padded to chunks of 128.
    segment_sum is computed per bucket as a sequence of PE matmuls:
       aggT[f, n] += sum_e xgath[e, f] * onehot[e, n]
    where xgath is an indirect-DMA gather of the source-node features and
    onehot[e, n] = vals[e] * (n == local_row[e]) is built on the vector
    engine from an iota constant.
  - x = node_emb + type_emb[type] is built redundantly on every core
    (original node order); h (layer-1 out) and z (layer-2 out) live in the
    permuted order, sharded by core and AllGathered.
  - Pairs are sharded by batch; the predictor gathers z rows, transposes on
    the PE, and runs the tiny MLP per 128-pair chunk.

Feature tables (x, h, z) are stored bf16; all matmuls run in bf16 with f32
PSUM accumulation.
"""

import numpy as np
import ml_dtypes

import concourse.bass as bass
import concourse.bacc as bacc
import concourse.tile as tile
import concourse.mybir as mybir
from concourse import bass_utils

PT = 128  # partitions
NCORES = 8
TE = 16  # padded type-embedding rows

BF16 = ml_dtypes.bfloat16

_prog_cache: dict = {}


def _preprocess(node_type_ids, adj_rows, adj_cols, adj_vals, pairs,
                node_emb, type_emb, W1, b1, W2, b2, Wp1, bp1, Wp2, bp2):
    N, H = node_emb.shape
    T = type_emb.shape[0]
    E = adj_rows.shape[0]
    P2 = pairs.shape[1]
    D = W2.shape[1]
    assert H == PT and T <= TE and Wp1.shape == (3 * D, D)

    NPB = PT * NCORES
    NPAD = -(-N // NPB) * NPB
    NBLK = NPAD // PT
    BPC = NBLK // NCORES

    rows = np.asarray(adj_rows).astype(np.int64)
    cols = np.asarray(adj_cols).astype(np.int64)
    vals = np.asarray(adj_vals).astype(np.float32)
    types = np.asarray(node_type_ids).astype(np.int64)

    # Degree-balanced bucket assignment: sort nodes by in-degree (desc) and
    # deal them round-robin across the NBLK buckets; bucket b, slot s ->
    # new node id b*128+s.
    deg = np.bincount(rows, minlength=N)
    order = np.argsort(-deg, kind="stable")
    i = np.arange(NPAD)
    newid_of_rank = (i % NBLK) * PT + (i // NBLK)
    perm = np.empty(N, np.int64)
    perm[order] = newid_of_rank[:N]

    prow = perm[rows]
    bkt = prow // PT
    rid = (prow % PT).astype(np.float32)
    cnt = np.bincount(bkt, minlength=NBLK)
    C = max(1, int(-(-int(cnt.max()) // PT)))
    CAP = C * PT

    eord = np.argsort(bkt, kind="stable")
    bs = bkt[eord]
    starts = np.concatenate([[0], np.cumsum(cnt)[:-1]])
    ps = np.arange(E) - starts[bs]

    ecol1 = np.zeros((NBLK, CAP), np.int32)
    ecol2 = np.zeros((NBLK, CAP), np.int32)
    erid = np.zeros((NBLK, CAP), np.float32)
    evalv = np.zeros((NBLK, CAP), np.float32)
    ce = cols[eord]
    ecol1[bs, ps] = ce
    ecol2[bs, ps] = perm[ce]
    erid[bs, ps] = rid[eord]
    evalv[bs, ps] = vals[eord]

    def per_core_T(a):
        # [NBLK, C*PT] -> per-core [PT, BPC*C]; column blk*C+c, partition p
        # holds bucket (core*BPC+blk) edge slot c*128+p.
        out = []
        for k in range(NCORES):
            sub = a[k * BPC:(k + 1) * BPC].reshape(BPC, C, PT)
            out.append(np.ascontiguousarray(
                sub.transpose(2, 0, 1).reshape(PT, BPC * C)))
        return out

    cols1_k = per_core_T(ecol1)
    cols2_k = per_core_T(ecol2)
    rid_k = per_core_T(erid)
    val_k = per_core_T(evalv)

    PPC = P2 // NCORES
    assert PPC % PT == 0
    PC = PPC // PT
    pp = perm[np.asarray(pairs).astype(np.int64)]
    psrc_k = [np.ascontiguousarray(
        pp[0, k * PPC:(k + 1) * PPC].reshape(PC, PT).T.astype(np.int32))
        for k in range(NCORES)]
    pdst_k = [np.ascontiguousarray(
        pp[1, k * PPC:(k + 1) * PPC].reshape(PC, PT).T.astype(np.int32))
        for k in range(NCORES)]

    types_pad = np.zeros(NPAD, np.int64)
    types_pad[:N] = types
    types_t = np.ascontiguousarray(types_pad.reshape(NBLK, PT).T.astype(np.int32))

    node_emb_pad = np.zeros((NPAD, H), np.float32)
    node_emb_pad[:N] = np.asarray(node_emb, np.float32)
    type_emb_pad = np.zeros((TE, H), np.float32)
    type_emb_pad[:T] = np.asarray(type_emb, np.float32)

    Wp1 = np.asarray(Wp1, np.float32)
    shared = dict(
        node_emb=node_emb_pad,
        type_emb=type_emb_pad,
        types_t=types_t,
        w1=np.asarray(W1, np.float32).astype(BF16),
        w2=np.asarray(W2, np.float32).astype(BF16),
        wp1a=np.ascontiguousarray(Wp1[0:D]).astype(BF16),
        wp1b=np.ascontiguousarray(Wp1[D:2 * D]).astype(BF16),
        wp1c=np.ascontiguousarray(Wp1[2 * D:3 * D]).astype(BF16),
        wp2=np.asarray(Wp2, np.float32).astype(BF16),
        b1bc=np.ascontiguousarray(np.broadcast_to(
            np.asarray(b1, np.float32), (PT, H))),
        b2bc=np.ascontiguousarray(np.broadcast_to(
            np.asarray(b2, np.float32), (PT, D))),
        bp1col=np.asarray(bp1, np.float32).reshape(D, 1),
        iota=np.ascontiguousarray(np.broadcast_to(
            np.arange(PT, dtype=np.float32), (PT, PT))).astype(BF16),
        ident=np.eye(PT, dtype=np.float32).astype(BF16),
    )
    per_core = [dict(cols1=cols1_k[k], cols2=cols2_k[k], ridt=rid_k[k],
                     valt=val_k[k], psrc=psrc_k[k], pdst=pdst_k[k])
                for k in range(NCORES)]
    meta = dict(NPAD=NPAD, NBLK=NBLK, BPC=BPC, C=C, PC=PC, H=H, D=D,
                bp2f=float(np.asarray(bp2).reshape(-1)[0]))
    return meta, shared, per_core


def _build(meta):
    NPAD, NBLK, BPC, C, PC = (meta["NPAD"], meta["NBLK"], meta["BPC"],
                              meta["C"], meta["PC"])
    H, D, bp2f = meta["H"], meta["D"], meta["bp2f"]
    f32, bf16, i32 = mybir.dt.float32, mybir.dt.bfloat16, mybir.dt.int32
    AF = mybir.ActivationFunctionType
    Alu = mybir.AluOpType
    RG = [list(range(NCORES))]

    nc = bacc.Bacc("TRN2", target_bir_lowering=False, debug=False,
                   num_devices=NCORES)

    # kernel I/O
    node_emb = nc.dram_tensor("node_emb", [NPAD, H], f32, kind="ExternalInput")
    type_emb = nc.dram_tensor("type_emb", [TE, H], f32, kind="ExternalInput")
    types_td = nc.dram_tensor("types_t", [PT, NBLK], i32, kind="ExternalInput")
    cols1_d = nc.dram_tensor("cols1", [PT, BPC * C], i32, kind="ExternalInput")
    cols2_d = nc.dram_tensor("cols2", [PT, BPC * C], i32, kind="ExternalInput")
    ridt_d = nc.dram_tensor("ridt", [PT, BPC * C], f32, kind="ExternalInput")
    valt_d = nc.dram_tensor("valt", [PT, BPC * C], f32, kind="ExternalInput")
    psrc_d = nc.dram_tensor("psrc", [PT, PC], i32, kind="ExternalInput")
    pdst_d = nc.dram_tensor("pdst", [PT, PC], i32, kind="ExternalInput")
    w1_d = nc.dram_tensor("w1", [H, H], bf16, kind="ExternalInput")
    w2_d = nc.dram_tensor("w2", [H, D], bf16, kind="ExternalInput")
    wp1a_d = nc.dram_tensor("wp1a", [D, D], bf16, kind="ExternalInput")
    wp1b_d = nc.dram_tensor("wp1b", [D, D], bf16, kind="ExternalInput")
    wp1c_d = nc.dram_tensor("wp1c", [D, D], bf16, kind="ExternalInput")
    wp2_d = nc.dram_tensor("wp2", [D, 1], bf16, kind="ExternalInput")
    b1bc_d = nc.dram_tensor("b1bc", [PT, H], f32, kind="ExternalInput")
    b2bc_d = nc.dram_tensor("b2bc", [PT, D], f32, kind="ExternalInput")
    bp1c_d = nc.dram_tensor("bp1col", [D, 1], f32, kind="ExternalInput")
    iota_d = nc.dram_tensor("iota", [PT, PT], bf16, kind="ExternalInput")
    ident_d = nc.dram_tensor("ident", [PT, PT], bf16, kind="ExternalInput")
    outp = nc.dram_tensor("out", [PC * PT, 1], f32, kind="ExternalOutput")

    # internal feature tables
    x_full = nc.dram_tensor("x_full", [NPAD, H], bf16, kind="Internal")
    h_shard = nc.dram_tensor("h_shard", [BPC * PT, H], bf16, kind="Internal")
    h_full = nc.dram_tensor("h_full", [NPAD, H], bf16, kind="Internal",
                            addr_space="Shared")
    z_shard = nc.dram_tensor("z_shard", [BPC * PT, D], bf16, kind="Internal")
    z_full = nc.dram_tensor("z_full", [NPAD, D], bf16, kind="Internal",
                            addr_space="Shared")

    with tile.TileContext(nc) as tc:
        with (
            tc.tile_pool(name="const", bufs=1) as cpool,
            tc.tile_pool(name="idx", bufs=1) as ipool,
            tc.tile_pool(name="xne", bufs=3) as xnp,
            tc.tile_pool(name="gath", bufs=6) as gpool,
            tc.tile_pool(name="onep", bufs=6) as opool,
            tc.tile_pool(name="accs", bufs=3) as apool,
            tc.tile_pool(name="outs", bufs=3) as hpool,
            tc.tile_pool(name="pred", bufs=4) as ppool,
            tc.tile_pool(name="ps_agg", bufs=2, space="PSUM") as ps_agg,
            tc.tile_pool(name="ps_out", bufs=2, space="PSUM") as ps_out,
            tc.tile_pool(name="ps_tr", bufs=2, space="PSUM") as ps_tr,
            tc.tile_pool(name="ps_y", bufs=2, space="PSUM") as ps_y,
        ):
            def sb(pool, dram, shape, dtype):
                t = pool.tile(shape, dtype, name=dram.name + "_sb")
                nc.sync.dma_start(t[:], dram[:])
                return t

            # resident SBUF state
            iota_sb = sb(cpool, iota_d, [PT, PT], bf16)
            ident_sb = sb(cpool, ident_d, [PT, PT], bf16)
            w1_sb = sb(cpool, w1_d, [H, H], bf16)
            w2_sb = sb(cpool, w2_d, [H, D], bf16)
            wp1a_sb = sb(cpool, wp1a_d, [D, D], bf16)
            wp1b_sb = sb(cpool, wp1b_d, [D, D], bf16)
            wp1c_sb = sb(cpool, wp1c_d, [D, D], bf16)
            wp2_sb = sb(cpool, wp2_d, [D, 1], bf16)
            b1bc_sb = sb(cpool, b1bc_d, [PT, H], f32)
            b2bc_sb = sb(cpool, b2bc_d, [PT, D], f32)
            bp1c_sb = sb(cpool, bp1c_d, [D, 1], f32)
            types_sb = sb(ipool, types_td, [PT, NBLK], i32)
            cols1_sb = sb(ipool, cols1_d, [PT, BPC * C], i32)
            cols2_sb = sb(ipool, cols2_d, [PT, BPC * C], i32)
            rid_sb = sb(ipool, ridt_d, [PT, BPC * C], f32)
            val_sb = sb(ipool, valt_d, [PT, BPC * C], f32)
            psrc_sb = sb(ipool, psrc_d, [PT, PC], i32)
            pdst_sb = sb(ipool, pdst_d, [PT, PC], i32)

            # ---- Phase X: x = node_emb + type_emb[type], all NPAD rows ----
            SB = 4  # blocks per supertile
            assert NBLK % SB == 0
            for st in range(NBLK // SB):
                a = st * SB * PT
                ne = xnp.tile([PT, SB * H], f32, name="ne")
                nc.sync.dma_start(
                    ne[:].rearrange("p (b f) -> p b f", b=SB),
                    node_emb[a:a + SB * PT, :].rearrange(
                        "(b p) f -> p b f", p=PT))
                xs = xnp.tile([PT, SB * H], bf16, name="xs")
                for b in range(SB):
                    blk = st * SB + b
                    tg = gpool.tile([PT, H], f32, name="tg")
                    nc.gpsimd.indirect_dma_start(
                        out=tg[:], out_offset=None, in_=type_emb[:],
                        in_offset=bass.IndirectOffsetOnAxis(
                            ap=types_sb[:, blk:blk + 1], axis=0))
                    nc.vector.tensor_add(
                        xs[:, b * H:(b + 1) * H], ne[:, b * H:(b + 1) * H],
                        tg[:])
                nc.sync.dma_start(
                    x_full[a:a + SB * PT, :].rearrange("(b p) f -> p b f", p=PT),
                    xs[:].rearrange("p (b f) -> p b f", b=SB))

            # ---- GCN layer: dst_shard[blk] = act(aggT.T @ W + bias) ----
            def gcn_layer(src_table, colsb, dst_shard, w_sb, bias_sb, Dout,
                          relu, suffix):
                for blk in range(BPC):
                    agg_ps = ps_agg.tile([PT, PT], f32, name="agg_ps")
                    for c in range(C):
                        g = blk * C + c
                        xg = gpool.tile([PT, H], bf16, name="xg" + suffix)
                        nc.gpsimd.indirect_dma_start(
                            out=xg[:], out_offset=None, in_=src_table[:],
                            in_offset=bass.IndirectOffsetOnAxis(
                                ap=colsb[:, g:g + 1], axis=0))
                        oh = opool.tile([PT, PT], bf16, name="oh" + suffix)
                        nc.vector.tensor_scalar(
                            oh[:], iota_sb[:], rid_sb[:, g:g + 1],
                            val_sb[:, g:g + 1], op0=Alu.is_equal, op1=Alu.mult)
                        nc.tensor.matmul(agg_ps[:], lhsT=xg[:], rhs=oh[:],
                                         start=(c == 0), stop=(c == C - 1))
                    aggT_sb = apool.tile([PT, PT], bf16, name="aggT" + suffix)
                    nc.vector.tensor_copy(aggT_sb[:], agg_ps[:])
                    o_ps = ps_out.tile([PT, Dout], f32, name="o_ps")
                    nc.tensor.matmul(o_ps[:], lhsT=aggT_sb[:], rhs=w_sb[:],
                                     start=True, stop=True)
                    o_sb = hpool.tile([PT, Dout], bf16, name="osb" + suffix)
                    if relu:
                        ob = hpool.tile([PT, Dout], f32, name="ob" + suffix)
                        nc.vector.tensor_add(ob[:], o_ps[:], bias_sb[:])
                        nc.scalar.activation(o_sb[:], ob[:], AF.Relu)
                    else:
                        nc.vector.tensor_add(o_sb[:], o_ps[:], bias_sb[:])
                    nc.sync.dma_start(
                        dst_shard[blk * PT:(blk + 1) * PT, :], o_sb[:])

            gcn_layer(x_full, cols1_sb, h_shard, w1_sb, b1bc_sb, H,
                      relu=True, suffix="1")
            nc.gpsimd.collective_compute(
                "AllGather", Alu.bypass, replica_groups=RG,
                ins=[h_shard[:]], outs=[h_full[:]])

            gcn_layer(h_full, cols2_sb, z_shard, w2_sb, b2bc_sb, D,
                      relu=False, suffix="2")
            nc.gpsimd.collective_compute(
                "AllGather", Alu.bypass, replica_groups=RG,
                ins=[z_shard[:]], outs=[z_full[:]])

            # ---- predictor ----
            for pc in range(PC):
                sg = ppool.tile([PT, D], bf16, name="sg")
                nc.gpsimd.indirect_dma_start(
                    out=sg[:], out_offset=None, in_=z_full[:],
                    in_offset=bass.IndirectOffsetOnAxis(
                        ap=psrc_sb[:, pc:pc + 1], axis=0))
                dg = ppool.tile([PT, D], bf16, name="dg")
                nc.gpsimd.indirect_dma_start(
                    out=dg[:], out_offset=None, in_=z_full[:],
                    in_offset=bass.IndirectOffsetOnAxis(
                        ap=pdst_sb[:, pc:pc + 1], axis=0))
                sgt_ps = ps_tr.tile([D, PT], bf16, name="sgt_ps", tag="tps")
                nc.tensor.transpose(sgt_ps[:], sg[:], ident_sb[:])
                dgt_ps = ps_tr.tile([D, PT], bf16, name="dgt_ps", tag="tps")
                nc.tensor.transpose(dgt_ps[:], dg[:], ident_sb[:])
                sgt = ppool.tile([D, PT], bf16, name="sgt")
                nc.vector.tensor_copy(sgt[:], sgt_ps[:])
                dgt = ppool.tile([D, PT], bf16, name="dgt")
                nc.vector.tensor_copy(dgt[:], dgt_ps[:])
                sdt = ppool.tile([D, PT], bf16, name="sdt")
                nc.vector.tensor_mul(sdt[:], sgt[:], dgt[:])
                yt_ps = ps_y.tile([D, PT], f32, name="yt_ps")
                nc.tensor.matmul(yt_ps[:], lhsT=wp1a_sb[:], rhs=sgt[:],
                                 start=True, stop=False)
                nc.tensor.matmul(yt_ps[:], lhsT=wp1b_sb[:], rhs=dgt[:],
                                 start=False, stop=False)
                nc.tensor.matmul(yt_ps[:], lhsT=wp1c_sb[:], rhs=sdt[:],
                                 start=False, stop=True)
                r_sb = ppool.tile([D, PT], bf16, name="r_sb")
                nc.scalar.activation(r_sb[:], yt_ps[:], AF.Relu,
                                     bias=bp1c_sb[:])
                o2_ps = ps_out.tile([PT, 1], f32, name="o2_ps", tag="o_ps")
                nc.tensor.matmul(o2_ps[:], lhsT=r_sb[:], rhs=wp2_sb[:],
                                 start=True, stop=True)
                o2_sb = ppool.tile([PT, 1], f32, name="o2_sb")
                nc.scalar.activation(o2_sb[:], o2_ps[:], AF.Copy, bias=bp2f)
                nc.sync.dma_start(outp[pc * PT:(pc + 1) * PT, :], o2_sb[:])

    nc.compile()
    return nc


def kernel(**inputs) -> np.ndarray:
    meta, shared, per_core = _preprocess(**inputs)
    key = tuple(sorted(meta.items()))
    if key not in _prog_cache:
        _prog_cache[key] = _build(meta)
    nc = _prog_cache[key]
    in_maps = [dict(shared, **per_core[k]) for k in range(NCORES)]
    res = bass_utils.run_bass_kernel_spmd(
        nc, in_maps, core_ids=list(range(NCORES)))
    out = np.concatenate(
        [np.asarray(res.results[k]["out"])[:, 0] for k in range(NCORES)])
    return out.astype(np.float32)


# revision 6
# speedup vs baseline: 1.1635x; 1.1635x over previous
"""Trainium2 Bass kernel for a 2-layer GCN + link predictor (PrimeKG drug
repurposing GNN).

Strategy (8 NeuronCores, SPMD single program):
  - Nodes are permuted into 128-node "buckets" balanced by in-degree; each
    core owns NBLK/8 consecutive buckets (rows of the aggregation).
  - Edges are grouped by destination bucket and padded to chunks of 128.
    segment_sum is computed per bucket as a sequence of PE matmuls:
       aggT[f, n] += sum_e xgath[e, f] * onehot[e, n]
    where xgath is an indirect-DMA gather of source-node features and
    onehot[e, n] = vals[e] * (n == local_row[e]) built on the vector engine.
  - x = node_emb + type_onehot.T @ type_emb is built sharded (original node
    order, host-precomputed transposed one-hot) and AllGathered; h and z
    live in permuted order, sharded and AllGathered likewise.
  - Pairs are sharded by batch; the predictor gathers z rows, transposes on
    the PE, and runs the tiny MLP per 128-pair chunk.

Feature tables and matmuls are fp16 with f32 PSUM accumulation.
"""

import numpy as np

import concourse.bass as bass
import concourse.bacc as bacc
import concourse.tile as tile
import concourse.mybir as mybir
from concourse import bass_utils

PT = 128  # partitions
NCORES = 8
TE = 16   # padded type-embedding rows

F16 = np.float16

_prog_cache: dict = {}


def _preprocess(node_type_ids, adj_rows, adj_cols, adj_vals, pairs,
                node_emb, type_emb, W1, b1, W2, b2, Wp1, bp1, Wp2, bp2):
    N, H = node_emb.shape
    T = type_emb.shape[0]
    E = adj_rows.shape[0]
    P2 = pairs.shape[1]
    D = W2.shape[1]
    assert H == PT and T <= TE and Wp1.shape == (3 * D, D)

    NPB = PT * NCORES
    NPAD = -(-N // NPB) * NPB
    NBLK = NPAD // PT
    BPC = NBLK // NCORES

    rows = np.asarray(adj_rows).astype(np.int64)
    cols = np.asarray(adj_cols).astype(np.int64)
    vals = np.asarray(adj_vals).astype(np.float32)
    types = np.asarray(node_type_ids).astype(np.int64)

    # Degree-balanced bucket assignment: deal nodes (sorted by in-degree
    # desc) round-robin across the NBLK buckets; new node id = bucket*128+slot.
    deg = np.bincount(rows, minlength=N)
    order = np.argsort(-deg, kind="stable")
    i = np.arange(NPAD)
    newid_of_rank = (i % NBLK) * PT + (i // NBLK)
    perm = np.empty(N, np.int64)
    perm[order] = newid_of_rank[:N]

    prow = perm[rows]
    bkt = prow // PT
    rid = (prow % PT).astype(np.float32)
    cnt = np.bincount(bkt, minlength=NBLK)
    C = max(1, int(-(-int(cnt.max()) // PT)))
    CAP = C * PT

    eord = np.argsort(bkt, kind="stable")
    bs = bkt[eord]
    starts = np.concatenate([[0], np.cumsum(cnt)[:-1]])
    ps = np.arange(E) - starts[bs]

    ecol1 = np.zeros((NBLK, CAP), np.int32)
    ecol2 = np.zeros((NBLK, CAP), np.int32)
    erid = np.zeros((NBLK, CAP), np.float32)
    evalv = np.zeros((NBLK, CAP), np.float32)
    ce = cols[eord]
    ecol1[bs, ps] = ce
    ecol2[bs, ps] = perm[ce]
    erid[bs, ps] = rid[eord]
    evalv[bs, ps] = vals[eord]

    def per_core_T(a):
        # [NBLK, C*PT] -> per-core [PT, BPC*C]; column blk*C+c, partition p
        # holds bucket (core*BPC+blk) edge slot c*128+p.
        out = []
        for k in range(NCORES):
            sub = a[k * BPC:(k + 1) * BPC].reshape(BPC, C, PT)
            out.append(np.ascontiguousarray(
                sub.transpose(2, 0, 1).reshape(PT, BPC * C)))
        return out

    cols1_k = per_core_T(ecol1)
    cols2_k = per_core_T(ecol2)
    rid_k = per_core_T(erid)
    val_k = per_core_T(evalv)

    PPC = P2 // NCORES
    assert PPC % PT == 0
    PC = PPC // PT
    pp = perm[np.asarray(pairs).astype(np.int64)]
    psrc_k = [np.ascontiguousarray(
        pp[0, k * PPC:(k + 1) * PPC].reshape(PC, PT).T.astype(np.int32))
        for k in range(NCORES)]
    pdst_k = [np.ascontiguousarray(
        pp[1, k * PPC:(k + 1) * PPC].reshape(PC, PT).T.astype(np.int32))
        for k in range(NCORES)]

    types_pad = np.zeros(NPAD, np.int64)
    types_pad[:N] = types
    types_oh_t = np.zeros((TE, NPAD), F16)  # transposed one-hot, exact 0/1
    types_oh_t[types_pad, np.arange(NPAD)] = 1.0

    node_emb_pad = np.zeros((NPAD, H), F16)
    node_emb_pad[:N] = np.asarray(node_emb, np.float32).astype(F16)
    type_emb_pad = np.zeros((TE, H), F16)
    type_emb_pad[:T] = np.asarray(type_emb, np.float32).astype(F16)

    SH = BPC * PT  # x-shard rows per core
    Wp1 = np.asarray(Wp1, np.float32)
    shared = dict(
        type_emb=type_emb_pad,
        w1=np.asarray(W1, np.float32).astype(F16),
        w2=np.asarray(W2, np.float32).astype(F16),
        wp1a=np.ascontiguousarray(Wp1[0:D]).astype(F16),
        wp1b=np.ascontiguousarray(Wp1[D:2 * D]).astype(F16),
        wp1c=np.ascontiguousarray(Wp1[2 * D:3 * D]).astype(F16),
        wp2=np.asarray(Wp2, np.float32).astype(F16),
        b1bc=np.ascontiguousarray(np.broadcast_to(
            np.asarray(b1, np.float32), (PT, H))),
        b2bc=np.ascontiguousarray(np.broadcast_to(
            np.asarray(b2, np.float32), (PT, D))),
        bp1col=np.asarray(bp1, np.float32).reshape(D, 1),
        iota=np.ascontiguousarray(np.broadcast_to(
            np.arange(PT, dtype=np.float32), (PT, PT))).astype(F16),
        ident=np.eye(PT, dtype=F16),
    )
    per_core = [dict(cols1=cols1_k[k], cols2=cols2_k[k], ridt=rid_k[k],
                     valt=val_k[k], psrc=psrc_k[k], pdst=pdst_k[k],
                     node_emb=np.ascontiguousarray(
                         node_emb_pad[k * SH:(k + 1) * SH]),
                     types_oh=np.ascontiguousarray(
                         types_oh_t[:, k * SH:(k + 1) * SH]))
                for k in range(NCORES)]
    meta = dict(NPAD=NPAD, NBLK=NBLK, BPC=BPC, C=C, PC=PC, H=H, D=D,
                bp2f=float(np.asarray(bp2).reshape(-1)[0]))
    return meta, shared, per_core


def _build(meta):
    NPAD, NBLK, BPC, C, PC = (meta["NPAD"], meta["NBLK"], meta["BPC"],
                              meta["C"], meta["PC"])
    H, D, bp2f = meta["H"], meta["D"], meta["bp2f"]
    f32, f16, i32 = mybir.dt.float32, mybir.dt.float16, mybir.dt.int32
    AF = mybir.ActivationFunctionType
    Alu = mybir.AluOpType
    RG = [list(range(NCORES))]
    SH = BPC * PT

    nc = bacc.Bacc("TRN2", target_bir_lowering=False, debug=False,
                   num_devices=NCORES)

    # kernel I/O
    node_emb = nc.dram_tensor("node_emb", [SH, H], f16, kind="ExternalInput")
    type_emb = nc.dram_tensor("type_emb", [TE, H], f16, kind="ExternalInput")
    tyoh_d = nc.dram_tensor("types_oh", [TE, SH], f16, kind="ExternalInput")
    cols1_d = nc.dram_tensor("cols1", [PT, BPC * C], i32, kind="ExternalInput")
    cols2_d = nc.dram_tensor("cols2", [PT, BPC * C], i32, kind="ExternalInput")
    ridt_d = nc.dram_tensor("ridt", [PT, BPC * C], f32, kind="ExternalInput")
    valt_d = nc.dram_tensor("valt", [PT, BPC * C], f32, kind="ExternalInput")
    psrc_d = nc.dram_tensor("psrc", [PT, PC], i32, kind="ExternalInput")
    pdst_d = nc.dram_tensor("pdst", [PT, PC], i32, kind="ExternalInput")
    w1_d = nc.dram_tensor("w1", [H, H], f16, kind="ExternalInput")
    w2_d = nc.dram_tensor("w2", [H, D], f16, kind="ExternalInput")
    wp1a_d = nc.dram_tensor("wp1a", [D, D], f16, kind="ExternalInput")
    wp1b_d = nc.dram_tensor("wp1b", [D, D], f16, kind="ExternalInput")
    wp1c_d = nc.dram_tensor("wp1c", [D, D], f16, kind="ExternalInput")
    wp2_d = nc.dram_tensor("wp2", [D, 1], f16, kind="ExternalInput")
    b1bc_d = nc.dram_tensor("b1bc", [PT, H], f32, kind="ExternalInput")
    b2bc_d = nc.dram_tensor("b2bc", [PT, D], f32, kind="ExternalInput")
    bp1c_d = nc.dram_tensor("bp1col", [D, 1], f32, kind="ExternalInput")
    iota_d = nc.dram_tensor("iota", [PT, PT], f16, kind="ExternalInput")
    ident_d = nc.dram_tensor("ident", [PT, PT], f16, kind="ExternalInput")
    outp = nc.dram_tensor("out", [PC * PT, 1], f32, kind="ExternalOutput")

    # internal feature tables
    x_shard = nc.dram_tensor("x_shard", [SH, H], f16, kind="Internal")
    x_full = nc.dram_tensor("x_full", [NPAD, H], f16, kind="Internal",
                            addr_space="Shared")
    h_shard = nc.dram_tensor("h_shard", [SH, H], f16, kind="Internal")
    h_full = nc.dram_tensor("h_full", [NPAD, H], f16, kind="Internal",
                            addr_space="Shared")
    z_shard = nc.dram_tensor("z_shard", [SH, D], f16, kind="Internal")
    z_full = nc.dram_tensor("z_full", [NPAD, D], f16, kind="Internal",
                            addr_space="Shared")

    with tile.TileContext(nc) as tc:
        with (
            tc.tile_pool(name="const", bufs=1) as cpool,
            tc.tile_pool(name="idx", bufs=1) as ipool,
            tc.tile_pool(name="xne", bufs=3) as xnp,
            tc.tile_pool(name="gath", bufs=6) as gpool,
            tc.tile_pool(name="onep", bufs=6) as opool,
            tc.tile_pool(name="accs", bufs=3) as apool,
            tc.tile_pool(name="outs", bufs=3) as hpool,
            tc.tile_pool(name="pred", bufs=4) as ppool,
            tc.tile_pool(name="ps_agg", bufs=2, space="PSUM") as ps_agg,
            tc.tile_pool(name="ps_out", bufs=2, space="PSUM") as ps_out,
            tc.tile_pool(name="ps_tr", bufs=2, space="PSUM") as ps_tr,
            tc.tile_pool(name="ps_y", bufs=2, space="PSUM") as ps_y,
        ):
            def sb(pool, dram, shape, dtype):
                t = pool.tile(shape, dtype, name=dram.name + "_sb")
                nc.sync.dma_start(t[:], dram[:])
                return t

            # resident SBUF state
            iota_sb = sb(cpool, iota_d, [PT, PT], f16)
            ident_sb = sb(cpool, ident_d, [PT, PT], f16)
            w1_sb = sb(cpool, w1_d, [H, H], f16)
            w2_sb = sb(cpool, w2_d, [H, D], f16)
            wp1a_sb = sb(cpool, wp1a_d, [D, D], f16)
            wp1b_sb = sb(cpool, wp1b_d, [D, D], f16)
            wp1c_sb = sb(cpool, wp1c_d, [D, D], f16)
            wp2_sb = sb(cpool, wp2_d, [D, 1], f16)
            b1bc_sb = sb(cpool, b1bc_d, [PT, H], f32)
            b2bc_sb = sb(cpool, b2bc_d, [PT, D], f32)
            bp1c_sb = sb(cpool, bp1c_d, [D, 1], f32)
            tyemb_sb = sb(cpool, type_emb, [TE, H], f16)
            tyoh_sb = sb(ipool, tyoh_d, [TE, SH], f16)
            cols1_sb = sb(ipool, cols1_d, [PT, BPC * C], i32)
            cols2_sb = sb(ipool, cols2_d, [PT, BPC * C], i32)
            rid_sb = sb(ipool, ridt_d, [PT, BPC * C], f32)
            val_sb = sb(ipool, valt_d, [PT, BPC * C], f32)
            psrc_sb = sb(ipool, psrc_d, [PT, PC], i32)
            pdst_sb = sb(ipool, pdst_d, [PT, PC], i32)

            # ---- Phase X (sharded): x = node_emb + types_oh.T @ type_emb ----
            SB = 7 if BPC % 7 == 0 else 1  # blocks per supertile
            assert BPC % SB == 0
            for st in range(BPC // SB):
                a = st * SB * PT
                ne = xnp.tile([PT, SB * H], f16, name="ne")
                nc.sync.dma_start(
                    ne[:].rearrange("p (b f) -> p b f", b=SB),
                    node_emb[a:a + SB * PT, :].rearrange(
                        "(b p) f -> p b f", p=PT))
                xs = xnp.tile([PT, SB * H], f16, name="xs")
                for b in range(SB):
                    blk = st * SB + b
                    t_ps = ps_out.tile([PT, H], f32, name="o_ps", tag="o_ps")
                    nc.tensor.matmul(
                        t_ps[:], lhsT=tyoh_sb[:, blk * PT:(blk + 1) * PT],
                        rhs=tyemb_sb[:], start=True, stop=True)
                    nc.vector.tensor_add(
                        xs[:, b * H:(b + 1) * H], ne[:, b * H:(b + 1) * H],
                        t_ps[:])
                nc.sync.dma_start(
                    x_shard[a:a + SB * PT, :].rearrange("(b p) f -> p b f", p=PT),
                    xs[:].rearrange("p (b f) -> p b f", b=SB))
            nc.gpsimd.collective_compute(
                "AllGather", Alu.bypass, replica_groups=RG,
                ins=[x_shard[:]], outs=[x_full[:]])

            # ---- GCN layer ----
            def gcn_layer(src_table, colsb, dst_shard, w_sb, bias_sb, Dout,
                          relu, suffix):
                for blk in range(BPC):
                    agg_ps = ps_agg.tile([PT, PT], f32, name="agg_ps")
                    for c in range(C):
                        g = blk * C + c
                        xg = gpool.tile([PT, H], f16, name="xg" + suffix)
                        nc.gpsimd.indirect_dma_start(
                            out=xg[:], out_offset=None, in_=src_table[:],
                            in_offset=bass.IndirectOffsetOnAxis(
                                ap=colsb[:, g:g + 1], axis=0))
                        oh = opool.tile([PT, PT], f16, name="oh" + suffix)
                        nc.vector.tensor_scalar(
                            oh[:], iota_sb[:], rid_sb[:, g:g + 1],
                            val_sb[:, g:g + 1], op0=Alu.is_equal, op1=Alu.mult)
                        nc.tensor.matmul(agg_ps[:], lhsT=xg[:], rhs=oh[:],
                                         start=(c == 0), stop=(c == C - 1))
                    aggT_sb = apool.tile([PT, PT], f16, name="aggT" + suffix)
                    nc.vector.tensor_copy(aggT_sb[:], agg_ps[:])
                    o_ps = ps_out.tile([PT, Dout], f32, name="o_ps", tag="o_ps")
                    nc.tensor.matmul(o_ps[:], lhsT=aggT_sb[:], rhs=w_sb[:],
                                     start=True, stop=True)
                    o_sb = hpool.tile([PT, Dout], f16, name="osb" + suffix)
                    if relu:
                        ob = hpool.tile([PT, Dout], f32, name="ob" + suffix)
                        nc.vector.tensor_add(ob[:], o_ps[:], bias_sb[:])
                        nc.scalar.activation(o_sb[:], ob[:], AF.Relu)
                    else:
                        nc.vector.tensor_add(o_sb[:], o_ps[:], bias_sb[:])
                    nc.sync.dma_start(
                        dst_shard[blk * PT:(blk + 1) * PT, :], o_sb[:])

            gcn_layer(x_full, cols1_sb, h_shard, w1_sb, b1bc_sb, H,
                      relu=True, suffix="1")
            nc.gpsimd.collective_compute(
                "AllGather", Alu.bypass, replica_groups=RG,
                ins=[h_shard[:]], outs=[h_full[:]])

            gcn_layer(h_full, cols2_sb, z_shard, w2_sb, b2bc_sb, D,
                      relu=False, suffix="2")
            nc.gpsimd.collective_compute(
                "AllGather", Alu.bypass, replica_groups=RG,
                ins=[z_shard[:]], outs=[z_full[:]])

            # ---- predictor ----
            for pc in range(PC):
                sg = ppool.tile([PT, D], f16, name="sg")
                nc.gpsimd.indirect_dma_start(
                    out=sg[:], out_offset=None, in_=z_full[:],
                    in_offset=bass.IndirectOffsetOnAxis(
                        ap=psrc_sb[:, pc:pc + 1], axis=0))
                dg = ppool.tile([PT, D], f16, name="dg")
                nc.gpsimd.indirect_dma_start(
                    out=dg[:], out_offset=None, in_=z_full[:],
                    in_offset=bass.IndirectOffsetOnAxis(
                        ap=pdst_sb[:, pc:pc + 1], axis=0))
                sgt_ps = ps_tr.tile([D, PT], f16, name="sgt_ps", tag="tps")
                nc.tensor.transpose(sgt_ps[:], sg[:], ident_sb[:])
                dgt_ps = ps_tr.tile([D, PT], f16, name="dgt_ps", tag="tps")
                nc.tensor.transpose(dgt_ps[:], dg[:], ident_sb[:])
                sgt = ppool.tile([D, PT], f16, name="sgt")
                nc.vector.tensor_copy(sgt[:], sgt_ps[:])
                dgt = ppool.tile([D, PT], f16, name="dgt")
                nc.vector.tensor_copy(dgt[:], dgt_ps[:])
                sdt = ppool.tile([D, PT], f16, name="sdt")
                nc.vector.tensor_mul(sdt[:], sgt[:], dgt[:])
                yt_ps = ps_y.tile([D, PT], f32, name="yt_ps")
                nc.tensor.matmul(yt_ps[:], lhsT=wp1a_sb[:], rhs=sgt[:],
                                 start=True, stop=False)
                nc.tensor.matmul(yt_ps[:], lhsT=wp1b_sb[:], rhs=dgt[:],
                                 start=False, stop=False)
                nc.tensor.matmul(yt_ps[:], lhsT=wp1c_sb[:], rhs=sdt[:],
                                 start=False, stop=True)
                r_sb = ppool.tile([D, PT], f16, name="r_sb")
                nc.scalar.activation(r_sb[:], yt_ps[:], AF.Relu,
                                     bias=bp1c_sb[:])
                o2_ps = ps_out.tile([PT, 1], f32, name="o2_ps", tag="o_ps")
                nc.tensor.matmul(o2_ps[:], lhsT=r_sb[:], rhs=wp2_sb[:],
                                 start=True, stop=True)
                o2_sb = ppool.tile([PT, 1], f32, name="o2_sb")
                nc.scalar.activation(o2_sb[:], o2_ps[:], AF.Copy, bias=bp2f)
                nc.sync.dma_start(outp[pc * PT:(pc + 1) * PT, :], o2_sb[:])

    nc.compile()
    return nc


def kernel(**inputs) -> np.ndarray:
    meta, shared, per_core = _preprocess(**inputs)
    key = tuple(sorted(meta.items()))
    if key not in _prog_cache:
        _prog_cache[key] = _build(meta)
    nc = _prog_cache[key]
    in_maps = [dict(shared, **per_core[k]) for k in range(NCORES)]
    res = bass_utils.run_bass_kernel_spmd(
        nc, in_maps, core_ids=list(range(NCORES)))
    out = np.concatenate(
        [np.asarray(res.results[k]["out"])[:, 0] for k in range(NCORES)])
    return out.astype(np.float32)


# revision 9
# speedup vs baseline: 1.2205x; 1.0490x over previous
"""Trainium2 Bass kernel for a 2-layer GCN + link predictor (PrimeKG drug
repurposing GNN).

Strategy (8 NeuronCores, SPMD single program):
  - Nodes are permuted into 128-node "buckets" balanced by in-degree; each
    core owns NBLK/8 consecutive buckets (rows of the aggregation).
  - Edges are grouped by destination bucket and padded to chunks of 128.
    segment_sum is computed per bucket as a sequence of PE matmuls:
       aggT[f, n] += sum_e xgath[e, f] * onehot[e, n]
    where xgath is an indirect-DMA gather of source-node features and
    onehot[e, n] = vals[e] * (n == local_row[e]) built on the vector engine.
  - x = node_emb + type_onehot.T @ type_emb is built sharded (original node
    order, host-precomputed transposed one-hot) and AllGathered; h and z
    live in permuted order, sharded and AllGathered likewise.
  - Pairs are sharded by batch; the predictor gathers z rows, transposes on
    the PE, and runs the tiny MLP per 128-pair chunk.

Feature tables and matmuls are fp16 with f32 PSUM accumulation.
"""

import numpy as np

import concourse.bass as bass
import concourse.bacc as bacc
import concourse.tile as tile
import concourse.mybir as mybir
from concourse import bass_utils

PT = 128  # partitions
NCORES = 8
TE = 16   # padded type-embedding rows

F16 = np.float16

_prog_cache: dict = {}


def _preprocess(node_type_ids, adj_rows, adj_cols, adj_vals, pairs,
                node_emb, type_emb, W1, b1, W2, b2, Wp1, bp1, Wp2, bp2):
    N, H = node_emb.shape
    T = type_emb.shape[0]
    E = adj_rows.shape[0]
    P2 = pairs.shape[1]
    D = W2.shape[1]
    assert H == PT and T <= TE and Wp1.shape == (3 * D, D)

    NPB = PT * NCORES
    NPAD = -(-N // NPB) * NPB
    NBLK = NPAD // PT
    BPC = NBLK // NCORES

    rows = np.asarray(adj_rows).astype(np.int64)
    cols = np.asarray(adj_cols).astype(np.int64)
    vals = np.asarray(adj_vals).astype(np.float32)
    types = np.asarray(node_type_ids).astype(np.int64)

    # Degree-balanced bucket assignment: deal nodes (sorted by in-degree
    # desc) round-robin across the NBLK buckets, then repair-swap nodes
    # between heavy and light buckets to pull the max bucket load down to
    # the next multiple-of-128 boundary.
    deg = np.bincount(rows, minlength=N).astype(np.int64)
    deg_pad = np.zeros(NPAD, np.int64)
    deg_pad[:N] = deg
    order = np.argsort(-deg_pad, kind="stable")
    i = np.arange(NPAD)
    bucket_of_rank = i % NBLK
    slot_of_rank = i // NBLK
    bucket_of = np.empty(NPAD, np.int64)
    bucket_of[order] = bucket_of_rank
    loads = np.bincount(bucket_of, weights=deg_pad, minlength=NBLK).astype(
        np.int64)
    target = max(PT, int(-(-int(loads.max()) // PT) - 1) * PT)
    members = [list(order[b::NBLK][::-1]) for b in range(NBLK)]  # asc degree
    for _ in range(2000):
        hb = int(np.argmax(loads))
        if loads[hb] <= target:
            break
        lb = int(np.argmin(loads))
        done = False
        for mi in range(len(members[hb]) - 1, -1, -1):
            m = members[hb][mi]
            for li, l in enumerate(members[lb]):
                delta = deg_pad[m] - deg_pad[l]
                if delta <= 0:
                    break
                if loads[lb] + delta <= target:
                    members[hb][mi], members[lb][li] = l, m
                    loads[hb] -= delta
                    loads[lb] += delta
                    done = True
                    break
            if done:
                break
        if not done:
            break
    perm = np.empty(N, np.int64)
    for b in range(NBLK):
        for s, m in enumerate(members[b]):
            if m < N:
                perm[m] = b * PT + s

    prow = perm[rows]
    bkt = prow // PT
    rid = (prow % PT).astype(np.float32)
    cnt = np.bincount(bkt, minlength=NBLK)
    C = max(1, int(-(-int(cnt.max()) // PT)))
    CAP = C * PT

    eord = np.argsort(bkt, kind="stable")
    bs = bkt[eord]
    starts = np.concatenate([[0], np.cumsum(cnt)[:-1]])
    ps = np.arange(E) - starts[bs]

    ecol1 = np.zeros((NBLK, CAP), np.int32)
    ecol2 = np.zeros((NBLK, CAP), np.int32)
    erid = np.zeros((NBLK, CAP), np.float32)
    evalv = np.zeros((NBLK, CAP), np.float32)
    ce = cols[eord]
    ecol1[bs, ps] = ce
    ecol2[bs, ps] = perm[ce]
    erid[bs, ps] = rid[eord]
    evalv[bs, ps] = vals[eord]

    def per_core_T(a):
        # [NBLK, C*PT] -> per-core [PT, BPC*C]; column blk*C+c, partition p
        # holds bucket (core*BPC+blk) edge slot c*128+p.
        out = []
        for k in range(NCORES):
            sub = a[k * BPC:(k + 1) * BPC].reshape(BPC, C, PT)
            out.append(np.ascontiguousarray(
                sub.transpose(2, 0, 1).reshape(PT, BPC * C)))
        return out

    cols1_k = per_core_T(ecol1)
    cols2_k = per_core_T(ecol2)
    rid_k = per_core_T(erid)
    val_k = per_core_T(evalv)

    PPC = P2 // NCORES
    assert PPC % PT == 0
    PC = PPC // PT
    pp = perm[np.asarray(pairs).astype(np.int64)]
    psrc_k = [np.ascontiguousarray(
        pp[0, k * PPC:(k + 1) * PPC].reshape(PC, PT).T.astype(np.int32))
        for k in range(NCORES)]
    pdst_k = [np.ascontiguousarray(
        pp[1, k * PPC:(k + 1) * PPC].reshape(PC, PT).T.astype(np.int32))
        for k in range(NCORES)]

    types_pad = np.zeros(NPAD, np.int64)
    types_pad[:N] = types
    types_oh_t = np.zeros((TE, NPAD), F16)  # transposed one-hot, exact 0/1
    types_oh_t[types_pad, np.arange(NPAD)] = 1.0

    node_emb_pad = np.zeros((NPAD, H), F16)
    node_emb_pad[:N] = np.asarray(node_emb, np.float32).astype(F16)
    type_emb_pad = np.zeros((TE, H), F16)
    type_emb_pad[:T] = np.asarray(type_emb, np.float32).astype(F16)

    SH = BPC * PT  # x-shard rows per core
    Wp1 = np.asarray(Wp1, np.float32)
    shared = dict(
        type_emb=type_emb_pad,
        w1=np.asarray(W1, np.float32).astype(F16),
        w2=np.asarray(W2, np.float32).astype(F16),
        wp1a=np.ascontiguousarray(Wp1[0:D]).astype(F16),
        wp1b=np.ascontiguousarray(Wp1[D:2 * D]).astype(F16),
        wp1c=np.ascontiguousarray(Wp1[2 * D:3 * D]).astype(F16),
        wp2=np.asarray(Wp2, np.float32).astype(F16),
        b1bc=np.ascontiguousarray(np.broadcast_to(
            np.asarray(b1, np.float32), (PT, H))),
        b2bc=np.ascontiguousarray(np.broadcast_to(
            np.asarray(b2, np.float32), (PT, D))),
        bp1col=np.asarray(bp1, np.float32).reshape(D, 1),
        iota=np.ascontiguousarray(np.broadcast_to(
            np.arange(PT, dtype=np.float32), (PT, PT))).astype(F16),
        ident=np.eye(PT, dtype=F16),
    )
    per_core = [dict(cols1=cols1_k[k], cols2=cols2_k[k], ridt=rid_k[k],
                     valt=val_k[k], psrc=psrc_k[k], pdst=pdst_k[k],
                     node_emb=np.ascontiguousarray(
                         node_emb_pad[k * SH:(k + 1) * SH]),
                     types_oh=np.ascontiguousarray(
                         types_oh_t[:, k * SH:(k + 1) * SH]))
                for k in range(NCORES)]
    meta = dict(NPAD=NPAD, NBLK=NBLK, BPC=BPC, C=C, PC=PC, H=H, D=D,
                bp2f=float(np.asarray(bp2).reshape(-1)[0]))
    return meta, shared, per_core


def _build(meta):
    NPAD, NBLK, BPC, C, PC = (meta["NPAD"], meta["NBLK"], meta["BPC"],
                              meta["C"], meta["PC"])
    H, D, bp2f = meta["H"], meta["D"], meta["bp2f"]
    f32, f16, i32 = mybir.dt.float32, mybir.dt.float16, mybir.dt.int32
    AF = mybir.ActivationFunctionType
    Alu = mybir.AluOpType
    RG = [list(range(NCORES))]
    SH = BPC * PT

    nc = bacc.Bacc("TRN2", target_bir_lowering=False, debug=False,
                   num_devices=NCORES)

    # kernel I/O
    node_emb = nc.dram_tensor("node_emb", [SH, H], f16, kind="ExternalInput")
    type_emb = nc.dram_tensor("type_emb", [TE, H], f16, kind="ExternalInput")
    tyoh_d = nc.dram_tensor("types_oh", [TE, SH], f16, kind="ExternalInput")
    cols1_d = nc.dram_tensor("cols1", [PT, BPC * C], i32, kind="ExternalInput")
    cols2_d = nc.dram_tensor("cols2", [PT, BPC * C], i32, kind="ExternalInput")
    ridt_d = nc.dram_tensor("ridt", [PT, BPC * C], f32, kind="ExternalInput")
    valt_d = nc.dram_tensor("valt", [PT, BPC * C], f32, kind="ExternalInput")
    psrc_d = nc.dram_tensor("psrc", [PT, PC], i32, kind="ExternalInput")
    pdst_d = nc.dram_tensor("pdst", [PT, PC], i32, kind="ExternalInput")
    w1_d = nc.dram_tensor("w1", [H, H], f16, kind="ExternalInput")
    w2_d = nc.dram_tensor("w2", [H, D], f16, kind="ExternalInput")
    wp1a_d = nc.dram_tensor("wp1a", [D, D], f16, kind="ExternalInput")
    wp1b_d = nc.dram_tensor("wp1b", [D, D], f16, kind="ExternalInput")
    wp1c_d = nc.dram_tensor("wp1c", [D, D], f16, kind="ExternalInput")
    wp2_d = nc.dram_tensor("wp2", [D, 1], f16, kind="ExternalInput")
    b1bc_d = nc.dram_tensor("b1bc", [PT, H], f32, kind="ExternalInput")
    b2bc_d = nc.dram_tensor("b2bc", [PT, D], f32, kind="ExternalInput")
    bp1c_d = nc.dram_tensor("bp1col", [D, 1], f32, kind="ExternalInput")
    iota_d = nc.dram_tensor("iota", [PT, PT], f16, kind="ExternalInput")
    ident_d = nc.dram_tensor("ident", [PT, PT], f16, kind="ExternalInput")
    outp = nc.dram_tensor("out", [PC * PT, 1], f32, kind="ExternalOutput")

    # internal feature tables
    x_shard = nc.dram_tensor("x_shard", [SH, H], f16, kind="Internal")
    x_full = nc.dram_tensor("x_full", [NPAD, H], f16, kind="Internal",
                            addr_space="Shared")
    h_shard = nc.dram_tensor("h_shard", [SH, H], f16, kind="Internal")
    h_full = nc.dram_tensor("h_full", [NPAD, H], f16, kind="Internal",
                            addr_space="Shared")
    z_shard = nc.dram_tensor("z_shard", [SH, D], f16, kind="Internal")
    z_full = nc.dram_tensor("z_full", [NPAD, D], f16, kind="Internal",
                            addr_space="Shared")

    with tile.TileContext(nc) as tc:
        with (
            tc.tile_pool(name="const", bufs=1) as cpool,
            tc.tile_pool(name="idx", bufs=1) as ipool,
            tc.tile_pool(name="xne", bufs=3) as xnp,
            tc.tile_pool(name="gath", bufs=8) as gpool,
            tc.tile_pool(name="onep", bufs=8) as opool,
            tc.tile_pool(name="accs", bufs=3) as apool,
            tc.tile_pool(name="outs", bufs=3) as hpool,
            tc.tile_pool(name="pred", bufs=4) as ppool,
            tc.tile_pool(name="ps_agg", bufs=3, space="PSUM") as ps_agg,
            tc.tile_pool(name="ps_out", bufs=3, space="PSUM") as ps_out,
        ):
            def sb(pool, dram, shape, dtype):
                t = pool.tile(shape, dtype, name=dram.name + "_sb")
                nc.sync.dma_start(t[:], dram[:])
                return t

            # resident SBUF state
            iota_sb = sb(cpool, iota_d, [PT, PT], f16)
            ident_sb = sb(cpool, ident_d, [PT, PT], f16)
            w1_sb = sb(cpool, w1_d, [H, H], f16)
            w2_sb = sb(cpool, w2_d, [H, D], f16)
            wp1a_sb = sb(cpool, wp1a_d, [D, D], f16)
            wp1b_sb = sb(cpool, wp1b_d, [D, D], f16)
            wp1c_sb = sb(cpool, wp1c_d, [D, D], f16)
            wp2_sb = sb(cpool, wp2_d, [D, 1], f16)
            b1bc_sb = sb(cpool, b1bc_d, [PT, H], f32)
            b2bc_sb = sb(cpool, b2bc_d, [PT, D], f32)
            bp1c_sb = sb(cpool, bp1c_d, [D, 1], f32)
            tyemb_sb = sb(cpool, type_emb, [TE, H], f16)
            tyoh_sb = sb(ipool, tyoh_d, [TE, SH], f16)
            cols1_sb = sb(ipool, cols1_d, [PT, BPC * C], i32)
            cols2_sb = sb(ipool, cols2_d, [PT, BPC * C], i32)
            rid_sb = sb(ipool, ridt_d, [PT, BPC * C], f32)
            val_sb = sb(ipool, valt_d, [PT, BPC * C], f32)
            psrc_sb = sb(ipool, psrc_d, [PT, PC], i32)
            pdst_sb = sb(ipool, pdst_d, [PT, PC], i32)

            # ---- Phase X (sharded): x = node_emb + types_oh.T @ type_emb ----
            SB = 7 if BPC % 7 == 0 else 1  # blocks per supertile
            assert BPC % SB == 0
            for st in range(BPC // SB):
                a = st * SB * PT
                ne = xnp.tile([PT, SB * H], f16, name="ne")
                nc.sync.dma_start(
                    ne[:].rearrange("p (b f) -> p b f", b=SB),
                    node_emb[a:a + SB * PT, :].rearrange(
                        "(b p) f -> p b f", p=PT))
                xs = xnp.tile([PT, SB * H], f16, name="xs")
                for b in range(SB):
                    blk = st * SB + b
                    t_ps = ps_out.tile([PT, H], f32, name="o_ps", tag="o_ps")
                    nc.tensor.matmul(
                        t_ps[:], lhsT=tyoh_sb[:, blk * PT:(blk + 1) * PT],
                        rhs=tyemb_sb[:], start=True, stop=True)
                    nc.vector.tensor_add(
                        xs[:, b * H:(b + 1) * H], ne[:, b * H:(b + 1) * H],
                        t_ps[:])
                nc.sync.dma_start(
                    x_shard[a:a + SB * PT, :].rearrange("(b p) f -> p b f", p=PT),
                    xs[:].rearrange("p (b f) -> p b f", b=SB))
            nc.gpsimd.collective_compute(
                "AllGather", Alu.bypass, replica_groups=RG,
                ins=[x_shard[:]], outs=[x_full[:]])

            # ---- GCN layer ----
            def gcn_layer(src_table, colsb, dst_shard, w_sb, bias_sb, Dout,
                          relu, suffix):
                for blk in range(BPC):
                    agg_ps = ps_agg.tile([PT, PT], f32, name="agg_ps")
                    for c in range(C):
                        g = blk * C + c
                        xg = gpool.tile([PT, H], f16, name="xg" + suffix)
                        nc.gpsimd.indirect_dma_start(
                            out=xg[:], out_offset=None, in_=src_table[:],
                            in_offset=bass.IndirectOffsetOnAxis(
                                ap=colsb[:, g:g + 1], axis=0))
                        oh = opool.tile([PT, PT], f16, name="oh" + suffix)
                        nc.vector.tensor_scalar(
                            oh[:], iota_sb[:], rid_sb[:, g:g + 1],
                            val_sb[:, g:g + 1], op0=Alu.is_equal, op1=Alu.mult)
                        nc.tensor.matmul(agg_ps[:], lhsT=xg[:], rhs=oh[:],
                                         start=(c == 0), stop=(c == C - 1))
                    aggT_sb = apool.tile([PT, PT], f16, name="aggT" + suffix)
                    nc.vector.tensor_copy(aggT_sb[:], agg_ps[:])
                    o_ps = ps_out.tile([PT, Dout], f32, name="o_ps", tag="o_ps")
                    nc.tensor.matmul(o_ps[:], lhsT=aggT_sb[:], rhs=w_sb[:],
                                     start=True, stop=True)
                    o_sb = hpool.tile([PT, Dout], f16, name="osb" + suffix)
                    if relu:
                        ob = hpool.tile([PT, Dout], f32, name="ob" + suffix)
                        nc.vector.tensor_add(ob[:], o_ps[:], bias_sb[:])
                        nc.scalar.activation(o_sb[:], ob[:], AF.Relu)
                    else:
                        nc.vector.tensor_add(o_sb[:], o_ps[:], bias_sb[:])
                    nc.sync.dma_start(
                        dst_shard[blk * PT:(blk + 1) * PT, :], o_sb[:])

            gcn_layer(x_full, cols1_sb, h_shard, w1_sb, b1bc_sb, H,
                      relu=True, suffix="1")
            nc.gpsimd.collective_compute(
                "AllGather", Alu.bypass, replica_groups=RG,
                ins=[h_shard[:]], outs=[h_full[:]])

            gcn_layer(h_full, cols2_sb, z_shard, w2_sb, b2bc_sb, D,
                      relu=False, suffix="2")
            nc.gpsimd.collective_compute(
                "AllGather", Alu.bypass, replica_groups=RG,
                ins=[z_shard[:]], outs=[z_full[:]])

            # ---- predictor ----
            for pc in range(PC):
                sg = ppool.tile([PT, D], f16, name="sg")
                nc.gpsimd.indirect_dma_start(
                    out=sg[:], out_offset=None, in_=z_full[:],
                    in_offset=bass.IndirectOffsetOnAxis(
                        ap=psrc_sb[:, pc:pc + 1], axis=0))
                dg = ppool.tile([PT, D], f16, name="dg")
                nc.gpsimd.indirect_dma_start(
                    out=dg[:], out_offset=None, in_=z_full[:],
                    in_offset=bass.IndirectOffsetOnAxis(
                        ap=pdst_sb[:, pc:pc + 1], axis=0))
                sgt_ps = ps_out.tile([D, PT], f16, name="sgt_ps", tag="o_ps")
                nc.tensor.transpose(sgt_ps[:], sg[:], ident_sb[:])
                dgt_ps = ps_out.tile([D, PT], f16, name="dgt_ps", tag="o_ps")
                nc.tensor.transpose(dgt_ps[:], dg[:], ident_sb[:])
                sgt = ppool.tile([D, PT], f16, name="sgt")
                nc.vector.tensor_copy(sgt[:], sgt_ps[:])
                dgt = ppool.tile([D, PT], f16, name="dgt")
                nc.vector.tensor_copy(dgt[:], dgt_ps[:])
                sdt = ppool.tile([D, PT], f16, name="sdt")
                nc.vector.tensor_mul(sdt[:], sgt[:], dgt[:])
                yt_ps = ps_agg.tile([D, PT], f32, name="yt_ps", tag="agg_ps")
                nc.tensor.matmul(yt_ps[:], lhsT=wp1a_sb[:], rhs=sgt[:],
                                 start=True, stop=False)
                nc.tensor.matmul(yt_ps[:], lhsT=wp1b_sb[:], rhs=dgt[:],
                                 start=False, stop=False)
                nc.tensor.matmul(yt_ps[:], lhsT=wp1c_sb[:], rhs=sdt[:],
                                 start=False, stop=True)
                r_sb = ppool.tile([D, PT], f16, name="r_sb")
                nc.scalar.activation(r_sb[:], yt_ps[:], AF.Relu,
                                     bias=bp1c_sb[:])
                o2_ps = ps_out.tile([PT, 1], f32, name="o2_ps", tag="o_ps")
                nc.tensor.matmul(o2_ps[:], lhsT=r_sb[:], rhs=wp2_sb[:],
                                 start=True, stop=True)
                o2_sb = ppool.tile([PT, 1], f32, name="o2_sb")
                nc.scalar.activation(o2_sb[:], o2_ps[:], AF.Copy, bias=bp2f)
                nc.sync.dma_start(outp[pc * PT:(pc + 1) * PT, :], o2_sb[:])

    nc.compile()
    return nc


def kernel(**inputs) -> np.ndarray:
    meta, shared, per_core = _preprocess(**inputs)
    key = tuple(sorted(meta.items()))
    if key not in _prog_cache:
        _prog_cache[key] = _build(meta)
    nc = _prog_cache[key]
    in_maps = [dict(shared, **per_core[k]) for k in range(NCORES)]
    res = bass_utils.run_bass_kernel_spmd(
        nc, in_maps, core_ids=list(range(NCORES)))
    out = np.concatenate(
        [np.asarray(res.results[k]["out"])[:, 0] for k in range(NCORES)])
    return out.astype(np.float32)
